# revision 24
# baseline (speedup 1.0000x reference)
"""GCN layer (gnn_message_passing) Trainium2 Bass kernel.

Problem: out[b,n,:] = relu( sum_r (mean_k padded[b, idx[b,r,n,k]]) @ W_r
                            + feat[b,n] @ W_self + bias )
  B=4, N=4096, D=O=128, R=4, K=16.

Strategy: shard (batch x N-half) across 8 cores -> no collectives.

Per core (b, h), project-then-gather:
  Prologue (PE):
    ptbl_r = padded @ (W_r/K)  [4097p, O] bf16 -- one ldweights per
    128-row tile of paddedT, one matmul streaming all 4 relations'
    kernels (512 cols) into PSUM; DVE copies to SBUF bf16; HWDGE
    writes each relation's table back to DRAM.
    selfm = feat @ W_self + bias  [NH, O] bf16 kept in SBUF.
  Main loop, per (chunk of 512 nodes, relation):
    - SWDGE dma_gather (transpose=False -> plain copy descriptors, safe
      to run concurrently) pulls projected rows [p, c, o]; queue_num=r
      puts each relation's descriptor generation on its own Q7 core
      pair, 4-way parallel. Stream order is k-outer (j = k*512 + n) so
      node n's K rows share partition n%128.
    - DVE tensor_reduce over k (stride 4*256B) -> msg_r [n%128, s, o].
    - DVE adds the 4 relations + selfm; ACT applies ReLU -> f32 out.
  (Transpose-mode gathers are NOT safe on multiple queues: their rows
  funnel through shared per-SDMA-engine XBAR staging, and interleaved
  packets from different queues corrupt the 16-row transpose groups.)
"""

import numpy as np
import ml_dtypes

import concourse.bacc as bacc
import concourse.mybir as mybir
from concourse.tile import TileContext
from concourse.bass_utils import run_bass_kernel_spmd

B, N, D = 4, 4096, 128
R, K, O = 4, 16, 128
NCORES = 8
NH = N // 2            # nodes per core
CHUNK = 512            # nodes per chunk
NCH = NH // CHUNK      # chunks per core
RJ = CHUNK * K         # idxs per relation-call (8192)
SEG = R * RJ // 16     # idx cols per chunk: 4*512 = 2048
NT = 33                # 128-row tiles covering the 4097-row table
PG = 11                # table tiles per writeback group
TROWS = NT * 128       # padded table rows (4224)
G_BUFS = 7
PE_CH = (2, 3)           # chunks computed on the tensor engine (dense counts)
GA_CH = tuple(c for c in range(NCH) if c not in PE_CH)
NPE = len(PE_CH)

_cache = {}


def _build():
    nc = bacc.Bacc("TRN2", num_swdge_queues=4)
    tblT = nc.dram_tensor("tblT", [128, TROWS], mybir.dt.bfloat16, kind="ExternalInput")
    tbl = nc.dram_tensor("tbl", [TROWS, D], mybir.dt.bfloat16, kind="ExternalInput")
    idxs = nc.dram_tensor("idxs", [128, NCH * SEG], mybir.dt.int16, kind="ExternalInput")
    amat = nc.dram_tensor("amat", [128, NPE, R, NT // PG, PG, CHUNK],
                          mybir.dt.bfloat16, kind="ExternalInput")
    w = nc.dram_tensor("w", [128, R + 2, O], mybir.dt.bfloat16, kind="ExternalInput")
    featT = nc.dram_tensor("featT", [128, NH], mybir.dt.bfloat16, kind="ExternalInput")
    out = nc.dram_tensor("out", [NH, O], mybir.dt.float32, kind="ExternalOutput")

    with TileContext(nc) as tc:
        with (
            tc.tile_pool(name="const", bufs=1) as cpool,
            tc.tile_pool(name="ptd", bufs=1, space="DRAM") as dpool,
            tc.tile_pool(name="idx", bufs=2) as ipool,
            tc.tile_pool(name="g", bufs=G_BUFS) as gpool,
            tc.tile_pool(name="agg", bufs=8) as apool,
            tc.tile_pool(name="sum", bufs=4) as spool,
            tc.tile_pool(name="pg", bufs=2) as pgpool,
            tc.tile_pool(name="osb", bufs=2) as opool,
            tc.tile_pool(name="am", bufs=4) as ampool,
            tc.tile_pool(name="agt", bufs=4) as atpool,
            tc.tile_pool(name="pp", bufs=4, space="PSUM") as prpool,
            tc.tile_pool(name="ps", bufs=2, space="PSUM") as pspool,
            tc.tile_pool(name="agp", bufs=2, space="PSUM") as atppool,
        ):
            w_sb = cpool.tile([128, R + 2, O], mybir.dt.bfloat16)
            nc.sync.dma_start(w_sb[:], w[:])
            tblT_sb = cpool.tile([128, TROWS], mybir.dt.bfloat16)
            nc.sync.dma_start(tblT_sb[:], tblT[:])
            feat_sb = cpool.tile([128, NH], mybir.dt.bfloat16)
            nc.sync.dma_start(feat_sb[:], featT[:])
            ones = cpool.tile([1, 128], mybir.dt.bfloat16)
            nc.vector.memset(ones[:], 1.0)

            # --- prologue: project the table through all relation kernels ---
            # Grouped staging: project PG tiles, copy wide (one copy per
            # tile, all relations at once), write each group back early so
            # gathers start as soon as the last group lands.
            ptbls = [
                dpool.tile([TROWS, O], mybir.dt.bfloat16, tag=f"pt{r}",
                           name=f"ptbl{r}")
                for r in range(R)
            ]
            pg_last = None
            for grp in range(NT // PG):
                pg_sb = pgpool.tile([128, PG, R, O], mybir.dt.bfloat16, tag="pg")
                pg_last = pg_sb
                for ti in range(PG):
                    t = grp * PG + ti
                    pp = prpool.tile([128, R, O], mybir.dt.float32)
                    nc.tensor.matmul(
                        pp[:], tblT_sb[:, t * 128:(t + 1) * 128], w_sb[:, 0:R, :],
                        start=True, stop=True,
                    )
                    with nc.allow_low_precision(reason="bf16 gather table"):
                        if ti % 2 == 0:
                            nc.vector.tensor_copy(pg_sb[:, ti, :, :], pp[:])
                        else:
                            nc.scalar.activation(
                                pg_sb[:, ti, :, :], pp[:],
                                mybir.ActivationFunctionType.Copy,
                            )
                for r in range(R):
                    nc.sync.dma_start(
                        ptbls[r][grp * PG * 128:(grp + 1) * PG * 128, :].rearrange(
                            "(t p) o -> p t o", p=128
                        ),
                        pg_sb[:, :, r, :],
                    )

            # --- prologue: self messages (feat @ W_self + bias) ---
            selfm_sb = cpool.tile([128, NH // 128, O], mybir.dt.bfloat16)
            for t in range(NH // 128):
                ps = pspool.tile([128, O], mybir.dt.float32, tag="selfps")
                nc.tensor.matmul(
                    ps[:], feat_sb[:, t * 128:(t + 1) * 128], w_sb[:, R, :],
                    start=True, stop=False,
                )
                nc.tensor.matmul(
                    ps[:], ones[:1, :], w_sb[0:1, R + 1, :],
                    start=False, stop=True,
                )
                with nc.allow_low_precision(reason="bf16 self msg"):
                    nc.scalar.activation(
                        selfm_sb[:, t, :], ps[:],
                        mybir.ActivationFunctionType.Copy,
                    )

            # --- main loop: gather-path chunks ---
            for ch in GA_CH:
                idx_sb = ipool.tile([128, SEG], mybir.dt.int16)
                nc.sync.dma_start(idx_sb[:], idxs[:, ch * SEG:(ch + 1) * SEG])

                aggs = []
                for r in range(R):
                    # Two k-half gathers (k 0-7 / k 8-15) per relation: halves
                    # the SDMA drain the first reduce has to wait for.
                    halves = []
                    for h in range(2):
                        g = gpool.tile([128, RJ // 256, D], mybir.dt.bfloat16,
                                       tag="g")
                        base_c = r * (RJ // 16) + h * (RJ // 32)
                        nc.gpsimd.dma_gather(
                            g[:], ptbls[r][:],
                            idx_sb[:, base_c:base_c + RJ // 32],
                            RJ // 2, RJ // 2, D, transpose=False,
                            single_packet=False, queue_num=r,
                        )
                        # stream j = k*512 + n: row j at [p=j%128, c=j//128],
                        # c = k*4 + s, node n = s*128 + p. Reduce over k.
                        hf = apool.tile([128, CHUNK // 128, O],
                                        mybir.dt.bfloat16, tag="hf")
                        with nc.allow_low_precision(reason="bf16 msg sums"):
                            nc.vector.tensor_reduce(
                                hf[:],
                                g[:].rearrange("p (k s) e -> p s e k", k=K // 2),
                                mybir.AxisListType.X,
                                mybir.AluOpType.add,
                            )
                        halves.append(hf)
                    aggf = apool.tile([128, CHUNK // 128, O], mybir.dt.bfloat16,
                                      tag="aggf")
                    with nc.allow_low_precision(reason="bf16 msg sums"):
                        nc.vector.tensor_tensor(
                            aggf[:], halves[0][:], halves[1][:],
                            mybir.AluOpType.add,
                        )
                    aggs.append(aggf)

                with nc.allow_low_precision(reason="bf16 msg sums"):
                    s01 = spool.tile([128, CHUNK // 128, O], mybir.dt.bfloat16,
                                     tag="s01")
                    nc.vector.tensor_tensor(
                        s01[:], aggs[0][:], aggs[1][:], mybir.AluOpType.add
                    )
                    s23 = spool.tile([128, CHUNK // 128, O], mybir.dt.bfloat16,
                                     tag="s23")
                    nc.vector.tensor_tensor(
                        s23[:], aggs[2][:], aggs[3][:], mybir.AluOpType.add
                    )
                    stot = spool.tile([128, CHUNK // 128, O], mybir.dt.bfloat16,
                                      tag="stot")
                    nc.vector.tensor_tensor(
                        stot[:], s01[:], s23[:], mybir.AluOpType.add
                    )
                    sall = spool.tile([128, CHUNK // 128, O], mybir.dt.bfloat16,
                                      tag="sall")
                    nc.vector.tensor_tensor(
                        sall[:],
                        stot[:],
                        selfm_sb[:, ch * (CHUNK // 128):(ch + 1) * (CHUNK // 128), :],
                        mybir.AluOpType.add,
                    )
                out_sb = opool.tile([128, CHUNK // 128, O], mybir.dt.float32)
                # ReLU on DVE, not ACT: the Scalar engine is in-order and a
                # chunk-gated activation there stalls the PE path's A-matrix
                # DMAs queued behind it.
                nc.vector.tensor_scalar_max(out_sb[:], sall[:], 0.0)
                nc.sync.dma_start(
                    out[ch * CHUNK:(ch + 1) * CHUNK, :].rearrange(
                        "(t p) o -> p t o", p=128
                    ),
                    out_sb[:],
                )

            # --- tensor-engine chunks: aggT = tbl.T @ A (dense counts),
            # then the standard projection. Runs concurrently with the
            # gather chunks on the otherwise-idle PE. DMAs go via the ACT
            # engine's HWDGE to keep the Sync queue free for idx loads.
            if NPE:
                tbn_sb = cpool.tile([128, NT, D], mybir.dt.bfloat16)
                # Gate: every PE-path matmul reads tbn_sb, so making its load
                # wait on the last prologue staging tile keeps the PE clear
                # for the table projection (which gates all gathers).
                nc.vector.tensor_copy(tbn_sb[:1, :1, :1], pg_last[:1, :1, :1, :1])
                nc.scalar.dma_start(
                    tbn_sb[:], tbl[:].rearrange("(t p) d -> p t d", p=128)
                )
                for i, ch in enumerate(PE_CH):
                    agTs = []
                    for r in range(R):
                        agp = atppool.tile([128, CHUNK], mybir.dt.float32)
                        for grp in range(NT // PG):
                            am = ampool.tile([128, PG, CHUNK], mybir.dt.bfloat16,
                                             tag="am")
                            nc.scalar.dma_start(am[:], amat[:, i, r, grp, :, :])
                            for ti in range(PG):
                                t = grp * PG + ti
                                nc.tensor.matmul(
                                    agp[:], tbn_sb[:, t, :], am[:, ti, :],
                                    start=(t == 0), stop=(t == NT - 1),
                                )
                        agt = atpool.tile([128, CHUNK], mybir.dt.bfloat16,
                                          tag="agt")
                        with nc.allow_low_precision(reason="bf16 agg"):
                            nc.scalar.activation(
                                agt[:], agp[:], mybir.ActivationFunctionType.Copy
                            )
                        agTs.append(agt)
                    out_sb = opool.tile([128, CHUNK // 128, O], mybir.dt.float32)
                    for t in range(CHUNK // 128):
                        ps = pspool.tile([128, O], mybir.dt.float32, tag="selfps")
                        sl = slice(t * 128, (t + 1) * 128)
                        fsl = slice(ch * CHUNK + t * 128, ch * CHUNK + (t + 1) * 128)
                        for r in range(R):
                            nc.tensor.matmul(
                                ps[:], agTs[r][:, sl], w_sb[:, r, :],
                                start=(r == 0), stop=False,
                            )
                        nc.tensor.matmul(
                            ps[:], feat_sb[:, fsl], w_sb[:, R, :],
                            start=False, stop=False,
                        )
                        nc.tensor.matmul(
                            ps[:], ones[:1, :], w_sb[0:1, R + 1, :],
                            start=False, stop=True,
                        )
                        nc.scalar.activation(
                            out_sb[:, t, :], ps[:], mybir.ActivationFunctionType.Relu
                        )
                    nc.sync.dma_start(
                        out[ch * CHUNK:(ch + 1) * CHUNK, :].rearrange(
                            "(t p) o -> p t o", p=128
                        ),
                        out_sb[:],
                    )

    nc.compile()
    return nc


def _prep_inputs(node_features, neighbor_indices, relation_kernels, self_kernel, bias):
    """Host-side shard/layout prep. Returns per-core input maps."""
    nf = np.asarray(node_features)
    idx = np.asarray(neighbor_indices)
    in_maps = []
    tblTs = []
    tbls = []
    for b in range(B):
        t = np.zeros((128, TROWS), dtype=ml_dtypes.bfloat16)
        t[:, 1:N + 1] = nf[b].astype(ml_dtypes.bfloat16).T
        tblTs.append(t)
        tn = np.zeros((TROWS, D), dtype=ml_dtypes.bfloat16)
        tn[1:N + 1] = nf[b].astype(ml_dtypes.bfloat16)
        tbls.append(tn)

    w = np.zeros((128, R + 2, O), dtype=ml_dtypes.bfloat16)
    for r in range(R):
        w[:, r, :] = (np.asarray(relation_kernels)[r] / K).astype(ml_dtypes.bfloat16)
    w[:, R, :] = np.asarray(self_kernel).astype(ml_dtypes.bfloat16)
    w[0, R + 1, :] = np.asarray(bias).astype(ml_dtypes.bfloat16)

    for c in range(NCORES):
        b, h = divmod(c, 2)
        base = h * NH
        cols = np.empty((16, NCH * SEG), dtype=np.int16)
        for ch in range(NCH):
            seg = np.empty((16, SEG), dtype=np.int16)
            for r in range(R):
                stream = idx[b, r, base + ch * CHUNK: base + (ch + 1) * CHUNK, :]
                # k-outer stream order: j = k*CHUNK + n
                stream = stream.T.reshape(-1).astype(np.int16)
                seg[:, r * (RJ // 16):(r + 1) * (RJ // 16)] = stream.reshape(-1, 16).T
            cols[:, ch * SEG:(ch + 1) * SEG] = seg
        featT = np.ascontiguousarray(
            nf[b, base:base + NH, :].astype(ml_dtypes.bfloat16).T
        )
        amat = np.empty((128, NPE, R, NT // PG, PG, CHUNK), dtype=ml_dtypes.bfloat16)
        node_rep = np.repeat(np.arange(CHUNK), K)
        for i, ch in enumerate(PE_CH):
            for r in range(R):
                A = np.zeros((TROWS, CHUNK), np.float32)
                st = idx[b, r, base + ch * CHUNK: base + (ch + 1) * CHUNK, :]
                np.add.at(A, (st.reshape(-1), node_rep), 1.0)
                At = A.reshape(NT, 128, CHUNK).transpose(1, 0, 2)
                amat[:, i, r] = At.reshape(128, NT // PG, PG, CHUNK).astype(
                    ml_dtypes.bfloat16
                )
        in_maps.append({
            "tblT": tblTs[b],
            "tbl": tbls[b],
            "idxs": np.tile(cols, (8, 1)),
            "amat": amat,
            "w": w,
            "featT": featT,
        })
    return in_maps


def _run(in_maps, **kw):
    if "nc" not in _cache:
        _cache["nc"] = _build()
    return run_bass_kernel_spmd(_cache["nc"], in_maps, core_ids=list(range(NCORES)), **kw)


def kernel(node_features, neighbor_indices, relation_kernels, self_kernel, bias):
    in_maps = _prep_inputs(node_features, neighbor_indices, relation_kernels,
                           self_kernel, bias)
    res = _run(in_maps)
    out = np.empty((B, N, O), dtype=np.float32)
    for c in range(NCORES):
        b, h = divmod(c, 2)
        out[b, h * NH:(h + 1) * NH, :] = res.results[c]["out"]
    return out


# revision 26
# speedup vs baseline: 1.1074x; 1.1074x over previous
"""GCN layer (gnn_message_passing) Trainium2 Bass kernel.

Problem: out[b,n,:] = relu( sum_r (mean_k padded[b, idx[b,r,n,k]]) @ W_r
                            + feat[b,n] @ W_self + bias )
  B=4, N=4096, D=O=128, R=4, K=16.

Strategy: shard (batch x N-half) across 8 cores -> no collectives.
Per core, the node-message work is split across two concurrent paths so
every engine contributes: gather chunks (SWDGE+DVE) and dense-count
chunks (PE).

  Prologue (PE):
    ptbl_r = padded @ (W_r/K)  [4097, O] bf16 -- one ldweights per
    128-row tile of paddedT, one matmul streaming all 4 relations'
    kernels (512 cols) into PSUM; DVE/ACT alternate the staging copies;
    HWDGE writes each relation's projected table back to DRAM in
    11-tile groups so gathers can start early.
    selfm = feat @ W_self + bias  [NH, O] bf16 kept in SBUF.
  Gather chunks (chunks 0-1), per relation:
    - Two k-half SWDGE dma_gathers (4096 idxs each, transpose=False ->
      plain copy descriptors, safe to run concurrently) pull projected
      rows [p, c, o]; queue_num=r puts each relation's descriptor
      generation on its own Q7 core pair, 4-way parallel. Stream order
      is k-outer (j = k*512 + n) so node n's K rows share partition
      n%128 and the reduce needs no transpose.
    - DVE tensor_reduce over k (stride 4*256B) per half, then adds the
      halves, the 4 relations and selfm; the ReLU also runs on DVE
      (an ACT-side relu would stall the PE path's A-matrix DMAs queued
      behind it on the in-order Scalar engine).
  PE chunks (chunks 2-3): aggT[d, n] = tbl.T @ A_r accumulated over 33
    row tiles in PSUM, where A_r [4224, 512] are host-built neighbor
    count matrices (exact in bf16); then the standard projection
    (aggs @ W_r + feat @ W_self + bias, ReLU on ACT). A tiles stream
    via the Scalar engine's HWDGE.

  Notes:
  - Transpose-mode gathers are NOT safe on multiple SWDGE queues: their
    rows funnel through shared per-SDMA-engine XBAR staging, and
    interleaved packets from different queues corrupt the 16-row
    transpose groups (seen as ~14% wrong rows). Row-copy mode is safe.
  - The PE path's first table load is gated on the last prologue
    staging tile so the table projection (which gates all gathers)
    keeps the PE to itself at the start.
"""

import numpy as np
import ml_dtypes

import concourse.bacc as bacc
import concourse.mybir as mybir
from concourse.tile import TileContext
from concourse.bass_utils import run_bass_kernel_spmd

B, N, D = 4, 4096, 128
R, K, O = 4, 16, 128
NCORES = 8
NH = N // 2            # nodes per core
CHUNK = 512            # nodes per chunk
NCH = NH // CHUNK      # chunks per core
RJ = CHUNK * K         # idxs per relation-call (8192)
SEG = R * RJ // 16     # idx cols per chunk: 4*512 = 2048
NT = 33                # 128-row tiles covering the 4097-row table
PG = 11                # table tiles per writeback group
TROWS = NT * 128       # padded table rows (4224)
G_BUFS = 9
PE_CH = (2, 3)           # chunks computed on the tensor engine (dense counts)
GA_CH = tuple(c for c in range(NCH) if c not in PE_CH)
NPE = len(PE_CH)

_cache = {}


def _build():
    nc = bacc.Bacc("TRN2", num_swdge_queues=4)
    tblT = nc.dram_tensor("tblT", [128, TROWS], mybir.dt.bfloat16, kind="ExternalInput")
    tbl = nc.dram_tensor("tbl", [TROWS, D], mybir.dt.bfloat16, kind="ExternalInput")
    idxs = nc.dram_tensor("idxs", [128, NCH * SEG], mybir.dt.int16, kind="ExternalInput")
    amat = nc.dram_tensor("amat", [128, NPE, R, NT // PG, PG, CHUNK],
                          mybir.dt.bfloat16, kind="ExternalInput")
    w = nc.dram_tensor("w", [128, R + 2, O], mybir.dt.bfloat16, kind="ExternalInput")
    featT = nc.dram_tensor("featT", [128, NH], mybir.dt.bfloat16, kind="ExternalInput")
    out = nc.dram_tensor("out", [NH, O], mybir.dt.float32, kind="ExternalOutput")

    with TileContext(nc) as tc:
        with (
            tc.tile_pool(name="const", bufs=1) as cpool,
            tc.tile_pool(name="ptd", bufs=1, space="DRAM") as dpool,
            tc.tile_pool(name="idx", bufs=2) as ipool,
            tc.tile_pool(name="g", bufs=G_BUFS) as gpool,
            tc.tile_pool(name="agg", bufs=8) as apool,
            tc.tile_pool(name="sum", bufs=4) as spool,
            tc.tile_pool(name="pg", bufs=2) as pgpool,
            tc.tile_pool(name="osb", bufs=2) as opool,
            tc.tile_pool(name="am", bufs=2) as ampool,
            tc.tile_pool(name="agt", bufs=4) as atpool,
            tc.tile_pool(name="pp", bufs=4, space="PSUM") as prpool,
            tc.tile_pool(name="ps", bufs=2, space="PSUM") as pspool,
            tc.tile_pool(name="agp", bufs=2, space="PSUM") as atppool,
        ):
            w_sb = cpool.tile([128, R + 2, O], mybir.dt.bfloat16)
            nc.sync.dma_start(w_sb[:], w[:])
            tblT_sb = cpool.tile([128, TROWS], mybir.dt.bfloat16)
            nc.sync.dma_start(tblT_sb[:], tblT[:])
            feat_sb = cpool.tile([128, NH], mybir.dt.bfloat16)
            nc.sync.dma_start(feat_sb[:], featT[:])
            ones = cpool.tile([1, 128], mybir.dt.bfloat16)
            nc.vector.memset(ones[:], 1.0)

            # --- prologue: project the table through all relation kernels ---
            # Grouped staging: project PG tiles, copy wide (one copy per
            # tile, all relations at once), write each group back early so
            # gathers start as soon as the last group lands.
            ptbls = [
                dpool.tile([TROWS, O], mybir.dt.bfloat16, tag=f"pt{r}",
                           name=f"ptbl{r}")
                for r in range(R)
            ]
            pg_last = None
            for grp in range(NT // PG):
                pg_sb = pgpool.tile([128, PG, R, O], mybir.dt.bfloat16, tag="pg")
                pg_last = pg_sb
                for ti in range(PG):
                    t = grp * PG + ti
                    pp = prpool.tile([128, R, O], mybir.dt.float32)
                    nc.tensor.matmul(
                        pp[:], tblT_sb[:, t * 128:(t + 1) * 128], w_sb[:, 0:R, :],
                        start=True, stop=True,
                    )
                    with nc.allow_low_precision(reason="bf16 gather table"):
                        if ti % 2 == 0:
                            nc.vector.tensor_copy(pg_sb[:, ti, :, :], pp[:])
                        else:
                            nc.scalar.activation(
                                pg_sb[:, ti, :, :], pp[:],
                                mybir.ActivationFunctionType.Copy,
                            )
                for r in range(R):
                    nc.sync.dma_start(
                        ptbls[r][grp * PG * 128:(grp + 1) * PG * 128, :].rearrange(
                            "(t p) o -> p t o", p=128
                        ),
                        pg_sb[:, :, r, :],
                    )

            # --- prologue: self messages (feat @ W_self + bias) ---
            selfm_sb = cpool.tile([128, NH // 128, O], mybir.dt.bfloat16)
            for t in range(NH // 128):
                ps = pspool.tile([128, O], mybir.dt.float32, tag="selfps")
                nc.tensor.matmul(
                    ps[:], feat_sb[:, t * 128:(t + 1) * 128], w_sb[:, R, :],
                    start=True, stop=False,
                )
                nc.tensor.matmul(
                    ps[:], ones[:1, :], w_sb[0:1, R + 1, :],
                    start=False, stop=True,
                )
                with nc.allow_low_precision(reason="bf16 self msg"):
                    nc.scalar.activation(
                        selfm_sb[:, t, :], ps[:],
                        mybir.ActivationFunctionType.Copy,
                    )

            # --- main loop: gather-path chunks ---
            for ch in GA_CH:
                idx_sb = ipool.tile([128, SEG], mybir.dt.int16)
                nc.sync.dma_start(idx_sb[:], idxs[:, ch * SEG:(ch + 1) * SEG])

                aggs = []
                for r in range(R):
                    # Two k-half gathers (k 0-7 / k 8-15) per relation: halves
                    # the SDMA drain the first reduce has to wait for.
                    halves = []
                    for h in range(2):
                        g = gpool.tile([128, RJ // 256, D], mybir.dt.bfloat16,
                                       tag="g")
                        base_c = r * (RJ // 16) + h * (RJ // 32)
                        nc.gpsimd.dma_gather(
                            g[:], ptbls[r][:],
                            idx_sb[:, base_c:base_c + RJ // 32],
                            RJ // 2, RJ // 2, D, transpose=False,
                            single_packet=False, queue_num=r,
                        )
                        # stream j = k*512 + n: row j at [p=j%128, c=j//128],
                        # c = k*4 + s, node n = s*128 + p. Reduce over k.
                        hf = apool.tile([128, CHUNK // 128, O],
                                        mybir.dt.bfloat16, tag="hf")
                        with nc.allow_low_precision(reason="bf16 msg sums"):
                            nc.vector.tensor_reduce(
                                hf[:],
                                g[:].rearrange("p (k s) e -> p s e k", k=K // 2),
                                mybir.AxisListType.X,
                                mybir.AluOpType.add,
                            )
                        halves.append(hf)
                    aggf = apool.tile([128, CHUNK // 128, O], mybir.dt.bfloat16,
                                      tag="aggf")
                    with nc.allow_low_precision(reason="bf16 msg sums"):
                        nc.vector.tensor_tensor(
                            aggf[:], halves[0][:], halves[1][:],
                            mybir.AluOpType.add,
                        )
                    aggs.append(aggf)

                with nc.allow_low_precision(reason="bf16 msg sums"):
                    s01 = spool.tile([128, CHUNK // 128, O], mybir.dt.bfloat16,
                                     tag="s01")
                    nc.vector.tensor_tensor(
                        s01[:], aggs[0][:], aggs[1][:], mybir.AluOpType.add
                    )
                    s23 = spool.tile([128, CHUNK // 128, O], mybir.dt.bfloat16,
                                     tag="s23")
                    nc.vector.tensor_tensor(
                        s23[:], aggs[2][:], aggs[3][:], mybir.AluOpType.add
                    )
                    stot = spool.tile([128, CHUNK // 128, O], mybir.dt.bfloat16,
                                      tag="stot")
                    nc.vector.tensor_tensor(
                        stot[:], s01[:], s23[:], mybir.AluOpType.add
                    )
                    sall = spool.tile([128, CHUNK // 128, O], mybir.dt.bfloat16,
                                      tag="sall")
                    nc.vector.tensor_tensor(
                        sall[:],
                        stot[:],
                        selfm_sb[:, ch * (CHUNK // 128):(ch + 1) * (CHUNK // 128), :],
                        mybir.AluOpType.add,
                    )
                out_sb = opool.tile([128, CHUNK // 128, O], mybir.dt.float32)
                # ReLU on DVE, not ACT: the Scalar engine is in-order and a
                # chunk-gated activation there stalls the PE path's A-matrix
                # DMAs queued behind it.
                nc.vector.tensor_scalar_max(out_sb[:], sall[:], 0.0)
                nc.sync.dma_start(
                    out[ch * CHUNK:(ch + 1) * CHUNK, :].rearrange(
                        "(t p) o -> p t o", p=128
                    ),
                    out_sb[:],
                )

            # --- tensor-engine chunks: aggT = tbl.T @ A (dense counts),
            # then the standard projection. Runs concurrently with the
            # gather chunks on the otherwise-idle PE. DMAs go via the ACT
            # engine's HWDGE to keep the Sync queue free for idx loads.
            if NPE:
                tbn_sb = cpool.tile([128, NT, D], mybir.dt.bfloat16)
                # Gate: every PE-path matmul reads tbn_sb, so making its load
                # wait on the last prologue staging tile keeps the PE clear
                # for the table projection (which gates all gathers).
                nc.vector.tensor_copy(tbn_sb[:1, :1, :1], pg_last[:1, :1, :1, :1])
                nc.scalar.dma_start(
                    tbn_sb[:], tbl[:].rearrange("(t p) d -> p t d", p=128)
                )
                for i, ch in enumerate(PE_CH):
                    agTs = []
                    for r in range(R):
                        agp = atppool.tile([128, CHUNK], mybir.dt.float32)
                        for grp in range(NT // PG):
                            am = ampool.tile([128, PG, CHUNK], mybir.dt.bfloat16,
                                             tag="am")
                            nc.scalar.dma_start(am[:], amat[:, i, r, grp, :, :])
                            for ti in range(PG):
                                t = grp * PG + ti
                                nc.tensor.matmul(
                                    agp[:], tbn_sb[:, t, :], am[:, ti, :],
                                    start=(t == 0), stop=(t == NT - 1),
                                )
                        agt = atpool.tile([128, CHUNK], mybir.dt.bfloat16,
                                          tag="agt")
                        with nc.allow_low_precision(reason="bf16 agg"):
                            nc.scalar.activation(
                                agt[:], agp[:], mybir.ActivationFunctionType.Copy
                            )
                        agTs.append(agt)
                    out_sb = opool.tile([128, CHUNK // 128, O], mybir.dt.float32)
                    for t in range(CHUNK // 128):
                        ps = pspool.tile([128, O], mybir.dt.float32, tag="selfps")
                        sl = slice(t * 128, (t + 1) * 128)
                        fsl = slice(ch * CHUNK + t * 128, ch * CHUNK + (t + 1) * 128)
                        for r in range(R):
                            nc.tensor.matmul(
                                ps[:], agTs[r][:, sl], w_sb[:, r, :],
                                start=(r == 0), stop=False,
                            )
                        nc.tensor.matmul(
                            ps[:], feat_sb[:, fsl], w_sb[:, R, :],
                            start=False, stop=False,
                        )
                        nc.tensor.matmul(
                            ps[:], ones[:1, :], w_sb[0:1, R + 1, :],
                            start=False, stop=True,
                        )
                        nc.scalar.activation(
                            out_sb[:, t, :], ps[:], mybir.ActivationFunctionType.Relu
                        )
                    nc.sync.dma_start(
                        out[ch * CHUNK:(ch + 1) * CHUNK, :].rearrange(
                            "(t p) o -> p t o", p=128
                        ),
                        out_sb[:],
                    )

    nc.compile()
    return nc


def _prep_inputs(node_features, neighbor_indices, relation_kernels, self_kernel, bias):
    """Host-side shard/layout prep. Returns per-core input maps."""
    nf = np.asarray(node_features)
    idx = np.asarray(neighbor_indices)
    in_maps = []
    tblTs = []
    tbls = []
    for b in range(B):
        t = np.zeros((128, TROWS), dtype=ml_dtypes.bfloat16)
        t[:, 1:N + 1] = nf[b].astype(ml_dtypes.bfloat16).T
        tblTs.append(t)
        tn = np.zeros((TROWS, D), dtype=ml_dtypes.bfloat16)
        tn[1:N + 1] = nf[b].astype(ml_dtypes.bfloat16)
        tbls.append(tn)

    w = np.zeros((128, R + 2, O), dtype=ml_dtypes.bfloat16)
    for r in range(R):
        w[:, r, :] = (np.asarray(relation_kernels)[r] / K).astype(ml_dtypes.bfloat16)
    w[:, R, :] = np.asarray(self_kernel).astype(ml_dtypes.bfloat16)
    w[0, R + 1, :] = np.asarray(bias).astype(ml_dtypes.bfloat16)

    for c in range(NCORES):
        b, h = divmod(c, 2)
        base = h * NH
        cols = np.empty((16, NCH * SEG), dtype=np.int16)
        for ch in range(NCH):
            seg = np.empty((16, SEG), dtype=np.int16)
            for r in range(R):
                stream = idx[b, r, base + ch * CHUNK: base + (ch + 1) * CHUNK, :]
                # k-outer stream order: j = k*CHUNK + n
                stream = stream.T.reshape(-1).astype(np.int16)
                seg[:, r * (RJ // 16):(r + 1) * (RJ // 16)] = stream.reshape(-1, 16).T
            cols[:, ch * SEG:(ch + 1) * SEG] = seg
        featT = np.ascontiguousarray(
            nf[b, base:base + NH, :].astype(ml_dtypes.bfloat16).T
        )
        amat = np.empty((128, NPE, R, NT // PG, PG, CHUNK), dtype=ml_dtypes.bfloat16)
        node_rep = np.repeat(np.arange(CHUNK), K)
        for i, ch in enumerate(PE_CH):
            for r in range(R):
                A = np.zeros((TROWS, CHUNK), np.float32)
                st = idx[b, r, base + ch * CHUNK: base + (ch + 1) * CHUNK, :]
                np.add.at(A, (st.reshape(-1), node_rep), 1.0)
                At = A.reshape(NT, 128, CHUNK).transpose(1, 0, 2)
                amat[:, i, r] = At.reshape(128, NT // PG, PG, CHUNK).astype(
                    ml_dtypes.bfloat16
                )
        in_maps.append({
            "tblT": tblTs[b],
            "tbl": tbls[b],
            "idxs": np.tile(cols, (8, 1)),
            "amat": amat,
            "w": w,
            "featT": featT,
        })
    return in_maps


def _run(in_maps, **kw):
    if "nc" not in _cache:
        _cache["nc"] = _build()
    return run_bass_kernel_spmd(_cache["nc"], in_maps, core_ids=list(range(NCORES)), **kw)


def kernel(node_features, neighbor_indices, relation_kernels, self_kernel, bias):
    in_maps = _prep_inputs(node_features, neighbor_indices, relation_kernels,
                           self_kernel, bias)
    res = _run(in_maps)
    out = np.empty((B, N, O), dtype=np.float32)
    for c in range(NCORES):
        b, h = divmod(c, 2)
        out[b, h * NH:(h + 1) * NH, :] = res.results[c]["out"]
    return out
